# revision 9
# baseline (speedup 1.0000x reference)
"""GPTQ 4-bit quantized linear (CaiQuantLinear) on 8 TRN2 NeuronCores.

Computes out = x @ dequant(qweight, scales, qzeros) + bias where
  x: (4, 2048, 4096) fp16, qweight: (512, 4096) int32 (8x 4-bit per word,
  packed along input features), scales: (32, 4096) fp16, qzeros: (32, 512)
  int32 (packed along output features), bias: (4096,) fp16.
  Groups are contiguous blocks of 128 input features (g_idx = arange//128).

Sharding: tensor-parallel column split over output features. Each of the 8
cores gets 512 output columns (its slice of qweight/scales/qzeros/bias) and
the full x (replicated). No collectives; the host concatenates the 8 column
slices.

v2: x is transposed on the host while marshaling the shards (xT [in, seq]),
so the device streams it with plain full-bandwidth DMA instead of the
single-ring xbar DMA-transpose that stalled the PE ~30% of the time in v1.
The matmul is flipped accordingly: dequantized weight k-tiles [128 in,
128 out] are the stationary operand, xT chunks [128 in, 512 seq] the moving
operand, accumulating outT [128 out, 512 seq] in PSUM over the 32 k-tiles.
The bias rides along as a per-partition scalar on the fp32 PSUM drain, and
the [512 out, 8192 seq] result is un-transposed on the host after gather.

v3: qweight and scales are also marshaled transposed on the host (pure
layout, the unpack/dequant math stays on device), removing the qword
PE-transposes from the prologue; the dequant waves are spread across
engines (nibble shifts on GpSimd, fused sub*scale on DVE, PSUM drains of
the weight transposes on ScalarE) so no single engine gates the wave rate
and the PE stays dense from the first wave on.

Per-core kernel:
  1. Load qT (host-transposed) [out, word-row]: nibble index varies along
     the free dim; GpSimd immediate-shift unpack + a fused per-partition
     (subtract zero, multiply scale) DVE tensor_scalar produce w^T fp16.
  2. PE-transpose w^T back to [input-feature, out] k-tiles, k8-major so
     k-tiles become ready in waves; fp16 weights stay resident in SBUF as
     [128, 32 k-tiles, 512 out]. Chunk-0 matmuls interleave with these
     waves using pre-allocated PSUM banks.
  3. Stream xT in 512-seq chunks (one 4.2MB strided DMA each, triple
     buffered); per chunk, 4 out-blocks x 32 k-tile matmuls accumulate in
     rotating PSUM banks; fp32 bias-add on the drain (split DVE/ScalarE),
     SWDGE stores.
"""

import sys

if "/opt/trn_rl_repo" not in sys.path:
    sys.path.insert(0, "/opt/trn_rl_repo")

import numpy as np

B, S, IN, OUT = 4, 2048, 4096, 4096
SEQ = B * S                      # 8192
NCORES = 8
OUT_S = OUT // NCORES            # 512 output columns per core
PACK = 8                         # int32 packs 8 nibbles
GSIZE = 128                      # group size == k-tile size
CHUNK = 512                      # seq per chunk (matmul moving free dim)

_CACHE = {}


def _build(seq, in_f, out_s, chunk):
    """Build + compile the per-core Bass program. All cores run the same
    NEFF on their own input slices (SPMD, no collectives)."""
    import concourse.bass as bass  # noqa: F401
    import concourse.mybir as mybir
    import concourse.tile as tile
    from concourse import bacc
    from concourse.masks import make_identity

    dt = mybir.dt
    op = mybir.AluOpType
    P = 128
    KT = in_f // P                # k-tiles (== groups) = 32
    QR = in_f // PACK             # qweight rows = 512
    OT = out_s // P               # 128-wide output blocks per core = 4
    NCH = seq // chunk            # seq chunks = 16

    nc = bacc.Bacc("TRN2", target_bir_lowering=False, debug=False,
                   num_devices=NCORES)

    # x arrives pre-transposed: [in_f, seq]
    x_d = nc.dram_tensor("x", (in_f, seq), dt.float16, kind="ExternalInput")
    # qweight arrives host-transposed: [out_s, QR]
    qw_d = nc.dram_tensor("qweight", (out_s, QR), dt.int32,
                          kind="ExternalInput")
    # scales arrive host-transposed: [out_s, KT]
    sc_d = nc.dram_tensor("scales", (out_s, KT), dt.float16,
                          kind="ExternalInput")
    qz_d = nc.dram_tensor("qzeros", (KT, out_s // PACK), dt.int32,
                          kind="ExternalInput")
    # bias arrives as [128, OT]: column ob holds bias[ob*128 : (ob+1)*128]
    b_d = nc.dram_tensor("bias", (P, OT), dt.float16, kind="ExternalInput")
    # out is produced transposed: [out_s, seq]; host un-transposes
    out_d = nc.dram_tensor("out", (out_s, seq), dt.float16,
                           kind="ExternalOutput")

    xT = x_d.ap()
    qw = qw_d.ap()
    scales = sc_d.ap()
    qzeros = qz_d.ap()
    bias = b_d.ap()
    out = out_d.ap()

    # [p, kt, m] view of xT: row kt*128+p, col m
    xv = xT.rearrange("(kt p) s -> p kt s", p=P)
    # [p, ot, r] views of the host-transposed qweight / scales
    qv = qw.rearrange("(ot p) r -> p ot r", p=P)
    sv = scales.rearrange("(ot p) g -> p ot g", p=P)

    with tile.TileContext(nc) as tc:
        with (
            tc.tile_pool(name="const", bufs=1) as const_pool,
            tc.tile_pool(name="w", bufs=1) as w_pool,
            tc.tile_pool(name="wti", bufs=4) as wti_pool,
            tc.tile_pool(name="wt16", bufs=4) as wt16_pool,
            tc.tile_pool(name="xt", bufs=3) as xt_pool,
            tc.tile_pool(name="ot", bufs=4) as out_pool,
            tc.tile_pool(name="ps", bufs=8, space="PSUM") as psum_pool,
        ):
            # ---- constants ----
            ident = const_pool.tile([P, P], dt.float16)
            make_identity(nc, ident)

            # chunk-0/1 xT: issued first so they load during the dequant
            # prologue, split into k-quarters so wave k8's matmuls only
            # wait for quarter k8 (~1MB) instead of the whole chunk.
            NW = KT // PACK          # 4 waves, 8 k-tiles each
            # tiny dequant inputs first on the fast HWDGE ring: the z path
            # is the longest dependency chain to the first PE op
            qz_sb = const_pool.tile([KT, out_s // PACK], dt.int32)
            nc.sync.dma_start(qz_sb, qzeros)
            s16 = const_pool.tile([P, OT, KT], dt.float16)
            nc.sync.dma_start(s16, sv)
            xb0 = xt_pool.tile([P, KT, chunk], dt.float16, tag="xt",
                               name="xb0")
            xb1 = xt_pool.tile([P, KT, chunk], dt.float16, tag="xt",
                               name="xb1")
            for k8 in range(NW):
                ks = slice(k8 * PACK, (k8 + 1) * PACK)
                nc.sync.dma_start(xb0[:, ks, :], xv[:, ks, 0:chunk])
                nc.sync.dma_start(
                    xb1[:, ks, :], xv[:, ks, chunk:2 * chunk])
            pss0 = [psum_pool.tile([P, chunk], dt.float32, tag="acc",
                                   name=f"ps0_{ob}") for ob in range(OT)]
            # chunk-1 rides along in the waves for 2 of its 4 out-blocks
            # (PSUM budget: 4 + 2 accumulators + 2 rotating pstB = 8)
            pss1 = [psum_pool.tile([P, chunk], dt.float32, tag="acc",
                                   name=f"ps1_{ob}") for ob in range(2)]

            # ---- dequantize weights ----
            # w_all[:, k, :]: k-tile k of fp16 weights, [128 in x out_s]
            w_all = w_pool.tile([P, KT, out_s], dt.float16)

            # qT (host-transposed): [128 out, OT, 512 word-rows]
            qTs = const_pool.tile([P, OT, QR], dt.int32)
            nc.gpsimd.dma_start(qTs, qv)
            bias16 = const_pool.tile([P, OT], dt.float16)
            nc.gpsimd.dma_start(bias16, bias)
            bias32 = const_pool.tile([P, OT], dt.float32)
            nc.vector.tensor_copy(bias32, bias16)
            sTf = const_pool.tile([P, OT, KT], dt.float32)
            nc.vector.tensor_copy(sTf, s16)
            sT = [sTf[:, ot, :] for ot in range(OT)]
            z_i = const_pool.tile([KT, out_s], dt.int32)
            z_iv = z_i.rearrange("g (c s) -> g c s", s=PACK)
            for s in range(PACK):
                nc.vector.tensor_scalar(
                    out=z_iv[:, :, s], in0=qz_sb, scalar1=4 * s, scalar2=0xF,
                    op0=op.logical_shift_right, op1=op.bitwise_and)
            z1_16 = const_pool.tile([KT, out_s], dt.float16)
            nc.vector.tensor_scalar_add(z1_16, z_i, 1.0)
            pzs = psum_pool.tile([P, OT * KT], dt.float16, tag="acc",
                                 name="pzs")
            for ot in range(OT):
                nc.tensor.transpose(
                    pzs[:, ot * KT:(ot + 1) * KT],
                    z1_16[:, ot * P:(ot + 1) * P], ident[:KT, :KT])
            zsf = const_pool.tile([P, OT * KT], dt.float32)
            nc.vector.tensor_copy(zsf, pzs)
            z1T = [zsf[:, ot * KT:(ot + 1) * KT] for ot in range(OT)]

            # unpack + dequant + final transpose, k8-major so k-tiles
            # become ready in waves; chunk-0/1 matmuls ride along
            for k8 in range(NW):
                for ot in range(OT):
                    # nibbles along the free dim: in-feature 8*wr + s
                    wT32 = wti_pool.tile([P, PACK * P], dt.int32, tag="wti")
                    wv_ = wT32.rearrange("p (w s) -> p w s", s=PACK)
                    w0 = k8 * (QR // NW)
                    for s in range(PACK):
                        nc.vector.tensor_scalar(
                            out=wv_[:, :, s],
                            in0=qTs[:, ot, w0:w0 + QR // NW],
                            scalar1=4 * s, scalar2=0xF,
                            op0=op.logical_shift_right, op1=op.bitwise_and)
                    # fused dequant per group: (w - (z+1)) * scale -> fp16
                    wT16 = wt16_pool.tile([P, PACK * P], dt.float16,
                                          tag="wt16")
                    for gg in range(PACK):
                        g = k8 * PACK + gg
                        nc.vector.tensor_scalar(
                            out=wT16[:, gg * P:(gg + 1) * P],
                            in0=wT32[:, gg * P:(gg + 1) * P],
                            scalar1=z1T[ot][:, g:g + 1],
                            scalar2=sT[ot][:, g:g + 1],
                            op0=op.subtract, op1=op.mult)
                    pstB = psum_pool.tile([P, PACK * P], dt.float16,
                                          tag="acc")
                    for kk in range(PACK):
                        nc.tensor.transpose(
                            pstB[:, kk * P:(kk + 1) * P],
                            wT16[:, kk * P:(kk + 1) * P], ident)
                    nc.scalar.copy(
                        w_all[:, k8 * PACK:(k8 + 1) * PACK,
                              ot * P:(ot + 1) * P],
                        pstB.rearrange("p (kk r) -> p kk r", r=P))
                # chunk-0 (all obs) + chunk-1 (obs 0-1) matmuls for this
                # wave's k-tiles: w stationary, xT moving
                for ob in range(OT):
                    for k in range(k8 * PACK, (k8 + 1) * PACK):
                        nc.tensor.matmul(
                            pss0[ob],
                            lhsT=w_all[:, k, ob * P:(ob + 1) * P],
                            rhs=xb0[:, k, :],
                            start=(k == 0), stop=(k == KT - 1))
                for ob in range(2):
                    for k in range(k8 * PACK, (k8 + 1) * PACK):
                        nc.tensor.matmul(
                            pss1[ob],
                            lhsT=w_all[:, k, ob * P:(ob + 1) * P],
                            rhs=xb1[:, k, :],
                            start=(k == 0), stop=(k == KT - 1))

            # drain chunk 0
            for ob in range(OT):
                o16 = out_pool.tile([P, chunk], dt.float16, tag="o16",
                                    name=f"o16_0_{ob}")
                if ob % 2 == 0:
                    nc.vector.tensor_scalar_add(
                        o16, pss0[ob], bias32[:, ob:ob + 1])
                else:
                    nc.scalar.add(o16, pss0[ob], bias32[:, ob:ob + 1])
                nc.scalar.dma_start(
                    out[ob * P:(ob + 1) * P, 0:chunk], o16)

            # chunk-1 wave-obs drains
            for ob in range(2):
                o16 = out_pool.tile([P, chunk], dt.float16, tag="o16",
                                    name=f"o16_1_{ob}")
                if ob % 2 == 0:
                    nc.vector.tensor_scalar_add(
                        o16, pss1[ob], bias32[:, ob:ob + 1])
                else:
                    nc.scalar.add(o16, pss1[ob], bias32[:, ob:ob + 1])
                nc.scalar.dma_start(
                    out[ob * P:(ob + 1) * P, chunk:2 * chunk], o16)

            # ---- main loop ----
            for cn in range(1, NCH):
                if cn == 1:
                    xb = xb1
                else:
                    xb = xt_pool.tile([P, KT, chunk], dt.float16, tag="xt")
                    nc.sync.dma_start(
                        xb, xv[:, :, cn * chunk:(cn + 1) * chunk])
                for ob in range(2 if cn == 1 else 0, OT):
                    ps = psum_pool.tile([P, chunk], dt.float32, tag="acc",
                                        name=f"ps_{cn}_{ob}")
                    for k in range(KT):
                        nc.tensor.matmul(
                            ps,
                            lhsT=w_all[:, k, ob * P:(ob + 1) * P],
                            rhs=xb[:, k, :],
                            start=(k == 0), stop=(k == KT - 1))
                    o16 = out_pool.tile([P, chunk], dt.float16, tag="o16")
                    if ob % 2 == 0:
                        nc.vector.tensor_scalar_add(
                            o16, ps, bias32[:, ob:ob + 1])
                    else:
                        nc.scalar.add(o16, ps, bias32[:, ob:ob + 1])
                    nc.scalar.dma_start(
                        out[ob * P:(ob + 1) * P,
                            cn * chunk:(cn + 1) * chunk], o16)

    nc.compile()
    return nc


def _get_program(seq, in_f, out_s, chunk):
    key = (seq, in_f, out_s, chunk)
    if key not in _CACHE:
        _CACHE[key] = _build(seq, in_f, out_s, chunk)
    return _CACHE[key]


def make_in_maps(x, qweight, scales, qzeros, bias):
    """Shard the full inputs for the 8 cores (host-side marshaling).
    x / qweight / scales are transposed here so the device never pays
    for transposes of its operands; all arithmetic stays on device."""
    x2 = np.asarray(x).reshape(SEQ, IN)
    xT = np.ascontiguousarray(x2.T)                      # [IN, SEQ]
    qweight = np.asarray(qweight)
    scales = np.asarray(scales)
    qzeros = np.asarray(qzeros)
    bias = np.asarray(bias)

    zcols = OUT_S // PACK
    in_maps = []
    for c in range(NCORES):
        o0 = c * OUT_S
        in_maps.append({
            "x": xT,
            "qweight": np.ascontiguousarray(
                qweight[:, o0:o0 + OUT_S].T),            # [OUT_S, QR]
            "scales": np.ascontiguousarray(
                scales[:, o0:o0 + OUT_S].T),             # [OUT_S, KT]
            "qzeros": np.ascontiguousarray(
                qzeros[:, c * zcols:(c + 1) * zcols]),
            "bias": np.ascontiguousarray(
                bias[o0:o0 + OUT_S].reshape(OUT_S // 128, 128).T),
        })
    return in_maps


def kernel(x, qweight, scales, qzeros, g_idx=None, bias=None, **_unused):
    """Full-input entry point: shards over 8 cores, runs on HW, gathers."""
    from concourse.bass_utils import run_bass_kernel_spmd

    nc = _get_program(SEQ, IN, OUT_S, CHUNK)
    in_maps = make_in_maps(x, qweight, scales, qzeros, bias)

    res = run_bass_kernel_spmd(nc, in_maps, core_ids=list(range(NCORES)))
    # each core returns outT [OUT_S, SEQ]; un-transpose + concat on host
    full = np.concatenate(
        [res.results[c]["out"].T for c in range(NCORES)], axis=1)
    return full.reshape(B, S, OUT).astype(np.float16)


# revision 10
# speedup vs baseline: 1.1939x; 1.1939x over previous
"""GPTQ 4-bit quantized linear (CaiQuantLinear) on 8 TRN2 NeuronCores.

Computes out = x @ dequant(qweight, scales, qzeros) + bias where
  x: (4, 2048, 4096) fp16, qweight: (512, 4096) int32 (8x 4-bit per word,
  packed along input features), scales: (32, 4096) fp16, qzeros: (32, 512)
  int32 (packed along output features), bias: (4096,) fp16.
  Groups are contiguous blocks of 128 input features (g_idx = arange//128).

Sharding: tensor-parallel column split over output features. Each of the 8
cores gets 512 output columns (its slice of qweight/scales/qzeros/bias) and
the full x (replicated). No collectives; the host concatenates the 8 column
slices.

v2: x is transposed on the host while marshaling the shards (xT [in, seq]),
so the device streams it with plain full-bandwidth DMA instead of the
single-ring xbar DMA-transpose that stalled the PE ~30% of the time in v1.
The matmul is flipped accordingly: dequantized weight k-tiles [128 in,
128 out] are the stationary operand, xT chunks [128 in, 512 seq] the moving
operand, accumulating outT [128 out, 512 seq] in PSUM over the 32 k-tiles.
The bias rides along as a per-partition scalar on the fp32 PSUM drain, and
the [512 out, 8192 seq] result is un-transposed on the host after gather.

v3: qweight and scales are also marshaled transposed on the host (pure
layout, the unpack/dequant math stays on device), removing the qword
PE-transposes from the prologue; the dequant waves are spread across
engines (nibble shifts on GpSimd, fused sub*scale on DVE, PSUM drains of
the weight transposes on ScalarE) so no single engine gates the wave rate
and the PE stays dense from the first wave on.

Per-core kernel:
  1. Load qT (host-transposed) [out, word-row]: nibble index varies along
     the free dim; GpSimd immediate-shift unpack + a fused per-partition
     (subtract zero, multiply scale) DVE tensor_scalar produce w^T fp16.
  2. PE-transpose w^T back to [input-feature, out] k-tiles, k8-major so
     k-tiles become ready in waves; fp16 weights stay resident in SBUF as
     [128, 32 k-tiles, 512 out]. Chunk-0 matmuls interleave with these
     waves using pre-allocated PSUM banks.
  3. Stream xT in 512-seq chunks (one 4.2MB strided DMA each, triple
     buffered); per chunk, 4 out-blocks x 32 k-tile matmuls accumulate in
     rotating PSUM banks; fp32 bias-add on the drain (split DVE/ScalarE),
     SWDGE stores.
"""

import sys

if "/opt/trn_rl_repo" not in sys.path:
    sys.path.insert(0, "/opt/trn_rl_repo")

import numpy as np

B, S, IN, OUT = 4, 2048, 4096, 4096
SEQ = B * S                      # 8192
NCORES = 8
OUT_S = OUT // NCORES            # 512 output columns per core
PACK = 8                         # int32 packs 8 nibbles
GSIZE = 128                      # group size == k-tile size
CHUNK = 512                      # seq per chunk (matmul moving free dim)

_CACHE = {}


def _build(seq, in_f, out_s, chunk):
    """Build + compile the per-core Bass program. All cores run the same
    NEFF on their own input slices (SPMD, no collectives)."""
    import concourse.bass as bass  # noqa: F401
    import concourse.mybir as mybir
    import concourse.tile as tile
    from concourse import bacc
    from concourse.masks import make_identity

    dt = mybir.dt
    op = mybir.AluOpType
    P = 128
    KT = in_f // P                # k-tiles (== groups) = 32
    QR = in_f // PACK             # qweight rows = 512
    OT = out_s // P               # 128-wide output blocks per core = 4
    NCH = seq // chunk            # seq chunks = 16

    nc = bacc.Bacc("TRN2", target_bir_lowering=False, debug=False,
                   num_devices=NCORES)

    # x arrives pre-transposed: [in_f, seq]
    x_d = nc.dram_tensor("x", (in_f, seq), dt.float16, kind="ExternalInput")
    # qweight arrives host-transposed: [out_s, QR]
    qw_d = nc.dram_tensor("qweight", (out_s, QR), dt.int32,
                          kind="ExternalInput")
    # scales arrive host-transposed: [out_s, KT]
    sc_d = nc.dram_tensor("scales", (out_s, KT), dt.float16,
                          kind="ExternalInput")
    qz_d = nc.dram_tensor("qzeros", (KT, out_s // PACK), dt.int32,
                          kind="ExternalInput")
    # bias arrives as [128, OT]: column ob holds bias[ob*128 : (ob+1)*128]
    b_d = nc.dram_tensor("bias", (P, OT), dt.float16, kind="ExternalInput")
    # out is produced transposed: [out_s, seq]; host un-transposes
    out_d = nc.dram_tensor("out", (out_s, seq), dt.float16,
                           kind="ExternalOutput")

    xT = x_d.ap()
    qw = qw_d.ap()
    scales = sc_d.ap()
    qzeros = qz_d.ap()
    bias = b_d.ap()
    out = out_d.ap()

    # [p, kt, m] view of xT: row kt*128+p, col m
    xv = xT.rearrange("(kt p) s -> p kt s", p=P)
    # [p, ot, r] views of the host-transposed qweight / scales
    qv = qw.rearrange("(ot p) r -> p ot r", p=P)
    sv = scales.rearrange("(ot p) g -> p ot g", p=P)

    with tile.TileContext(nc) as tc:
        with (
            tc.tile_pool(name="const", bufs=1) as const_pool,
            tc.tile_pool(name="w", bufs=1) as w_pool,
            tc.tile_pool(name="wti", bufs=4) as wti_pool,
            tc.tile_pool(name="wt16", bufs=4) as wt16_pool,
            tc.tile_pool(name="xt", bufs=3) as xt_pool,
            tc.tile_pool(name="ot", bufs=6) as out_pool,
            tc.tile_pool(name="ps", bufs=8, space="PSUM") as psum_pool,
        ):
            # ---- constants ----
            ident = const_pool.tile([P, P], dt.float16)
            make_identity(nc, ident)

            # chunk-0/1 xT: issued first so they load during the dequant
            # prologue, split into k-quarters so wave k8's matmuls only
            # wait for quarter k8 (~1MB) instead of the whole chunk.
            NW = KT // PACK          # 4 waves, 8 k-tiles each
            # tiny dequant inputs first on the fast HWDGE ring: the z path
            # is the longest dependency chain to the first PE op
            qz_sb = const_pool.tile([KT, out_s // PACK], dt.int32)
            nc.sync.dma_start(qz_sb, qzeros)
            s16 = const_pool.tile([P, OT, KT], dt.float16)
            nc.sync.dma_start(s16, sv)
            xb0 = xt_pool.tile([P, KT, chunk], dt.float16, tag="xt",
                               name="xb0")
            xb1 = xt_pool.tile([P, KT, chunk], dt.float16, tag="xt",
                               name="xb1")
            for k8 in range(NW):
                ks = slice(k8 * PACK, (k8 + 1) * PACK)
                nc.sync.dma_start(xb0[:, ks, :], xv[:, ks, 0:chunk])
                nc.sync.dma_start(
                    xb1[:, ks, :], xv[:, ks, chunk:2 * chunk])
            pss0 = [psum_pool.tile([P, chunk], dt.float32, tag="acc",
                                   name=f"ps0_{ob}") for ob in range(OT)]
            # chunk-1 rides along in the waves for 2 of its 4 out-blocks
            # (PSUM budget: 4 + 2 accumulators + 2 rotating pstB = 8)
            pss1 = [psum_pool.tile([P, chunk], dt.float32, tag="acc",
                                   name=f"ps1_{ob}") for ob in range(2)]

            # ---- dequantize weights ----
            # w_all[:, k, :]: k-tile k of fp16 weights, [128 in x out_s]
            w_all = w_pool.tile([P, KT, out_s], dt.float16)

            # qT (host-transposed): [128 out, OT, 512 word-rows]
            qTs = const_pool.tile([P, OT, QR], dt.int32)
            nc.gpsimd.dma_start(qTs, qv)
            bias16 = const_pool.tile([P, OT], dt.float16)
            nc.gpsimd.dma_start(bias16, bias)
            bias32 = const_pool.tile([P, OT], dt.float32)
            nc.vector.tensor_copy(bias32, bias16)
            sTf = const_pool.tile([P, OT, KT], dt.float32)
            nc.vector.tensor_copy(sTf, s16)
            sT = [sTf[:, ot, :] for ot in range(OT)]
            z_i = const_pool.tile([KT, out_s], dt.int32)
            z_iv = z_i.rearrange("g (c s) -> g c s", s=PACK)
            for s in range(PACK):
                nc.vector.tensor_scalar(
                    out=z_iv[:, :, s], in0=qz_sb, scalar1=4 * s, scalar2=0xF,
                    op0=op.logical_shift_right, op1=op.bitwise_and)
            z1_16 = const_pool.tile([KT, out_s], dt.float16)
            nc.vector.tensor_scalar_add(z1_16, z_i, 1.0)
            pzs = psum_pool.tile([P, OT * KT], dt.float16, tag="acc",
                                 name="pzs")
            for ot in range(OT):
                nc.tensor.transpose(
                    pzs[:, ot * KT:(ot + 1) * KT],
                    z1_16[:, ot * P:(ot + 1) * P], ident[:KT, :KT])
            zsf = const_pool.tile([P, OT * KT], dt.float32)
            nc.vector.tensor_copy(zsf, pzs)
            z1T = [zsf[:, ot * KT:(ot + 1) * KT] for ot in range(OT)]

            # unpack + dequant + final transpose, k8-major so k-tiles
            # become ready in waves; chunk-0/1 matmuls ride along
            for k8 in range(NW):
                for ot in range(OT):
                    # nibbles along the free dim: in-feature 8*wr + s
                    wT32 = wti_pool.tile([P, PACK * P], dt.int32, tag="wti")
                    wv_ = wT32.rearrange("p (w s) -> p w s", s=PACK)
                    w0 = k8 * (QR // NW)
                    for s in range(PACK):
                        nc.vector.tensor_scalar(
                            out=wv_[:, :, s],
                            in0=qTs[:, ot, w0:w0 + QR // NW],
                            scalar1=4 * s, scalar2=0xF,
                            op0=op.logical_shift_right, op1=op.bitwise_and)
                    # fused dequant per group: (w - (z+1)) * scale -> fp16
                    wT16 = wt16_pool.tile([P, PACK * P], dt.float16,
                                          tag="wt16")
                    for gg in range(PACK):
                        g = k8 * PACK + gg
                        nc.vector.tensor_scalar(
                            out=wT16[:, gg * P:(gg + 1) * P],
                            in0=wT32[:, gg * P:(gg + 1) * P],
                            scalar1=z1T[ot][:, g:g + 1],
                            scalar2=sT[ot][:, g:g + 1],
                            op0=op.subtract, op1=op.mult)
                    pstB = psum_pool.tile([P, PACK * P], dt.float16,
                                          tag="acc")
                    for kk in range(PACK):
                        nc.tensor.transpose(
                            pstB[:, kk * P:(kk + 1) * P],
                            wT16[:, kk * P:(kk + 1) * P], ident)
                    nc.scalar.copy(
                        w_all[:, k8 * PACK:(k8 + 1) * PACK,
                              ot * P:(ot + 1) * P],
                        pstB.rearrange("p (kk r) -> p kk r", r=P))
                # chunk-0 (all obs) + chunk-1 (obs 0-1) matmuls for this
                # wave's k-tiles: w stationary, xT moving
                for ob in range(OT):
                    for k in range(k8 * PACK, (k8 + 1) * PACK):
                        nc.tensor.matmul(
                            pss0[ob],
                            lhsT=w_all[:, k, ob * P:(ob + 1) * P],
                            rhs=xb0[:, k, :],
                            start=(k == 0), stop=(k == KT - 1))
                for ob in range(2):
                    for k in range(k8 * PACK, (k8 + 1) * PACK):
                        nc.tensor.matmul(
                            pss1[ob],
                            lhsT=w_all[:, k, ob * P:(ob + 1) * P],
                            rhs=xb1[:, k, :],
                            start=(k == 0), stop=(k == KT - 1))

            # drain chunk 0
            for ob in range(OT):
                o16 = out_pool.tile([P, chunk], dt.float16, tag="o16",
                                    name=f"o16_0_{ob}")
                nc.vector.tensor_scalar_add(
                    o16, pss0[ob], bias32[:, ob:ob + 1])
                nc.scalar.dma_start(
                    out[ob * P:(ob + 1) * P, 0:chunk], o16)

            # chunk-1 wave-obs drains
            for ob in range(2):
                o16 = out_pool.tile([P, chunk], dt.float16, tag="o16",
                                    name=f"o16_1_{ob}")
                nc.vector.tensor_scalar_add(
                    o16, pss1[ob], bias32[:, ob:ob + 1])
                nc.scalar.dma_start(
                    out[ob * P:(ob + 1) * P, chunk:2 * chunk], o16)

            # ---- main loop ----
            for cn in range(1, NCH):
                if cn == 1:
                    xb = xb1
                else:
                    xb = xt_pool.tile([P, KT, chunk], dt.float16, tag="xt")
                    nc.sync.dma_start(
                        xb, xv[:, :, cn * chunk:(cn + 1) * chunk])
                for ob in range(2 if cn == 1 else 0, OT):
                    ps = psum_pool.tile([P, chunk], dt.float32, tag="acc",
                                        name=f"ps_{cn}_{ob}")
                    for k in range(KT):
                        nc.tensor.matmul(
                            ps,
                            lhsT=w_all[:, k, ob * P:(ob + 1) * P],
                            rhs=xb[:, k, :],
                            start=(k == 0), stop=(k == KT - 1))
                    o16 = out_pool.tile([P, chunk], dt.float16, tag="o16")
                    nc.vector.tensor_scalar_add(
                        o16, ps, bias32[:, ob:ob + 1])
                    nc.scalar.dma_start(
                        out[ob * P:(ob + 1) * P,
                            cn * chunk:(cn + 1) * chunk], o16)

    nc.compile()
    return nc


def _get_program(seq, in_f, out_s, chunk):
    key = (seq, in_f, out_s, chunk)
    if key not in _CACHE:
        _CACHE[key] = _build(seq, in_f, out_s, chunk)
    return _CACHE[key]


def make_in_maps(x, qweight, scales, qzeros, bias):
    """Shard the full inputs for the 8 cores (host-side marshaling).
    x / qweight / scales are transposed here so the device never pays
    for transposes of its operands; all arithmetic stays on device."""
    x2 = np.asarray(x).reshape(SEQ, IN)
    xT = np.ascontiguousarray(x2.T)                      # [IN, SEQ]
    qweight = np.asarray(qweight)
    scales = np.asarray(scales)
    qzeros = np.asarray(qzeros)
    bias = np.asarray(bias)

    zcols = OUT_S // PACK
    in_maps = []
    for c in range(NCORES):
        o0 = c * OUT_S
        in_maps.append({
            "x": xT,
            "qweight": np.ascontiguousarray(
                qweight[:, o0:o0 + OUT_S].T),            # [OUT_S, QR]
            "scales": np.ascontiguousarray(
                scales[:, o0:o0 + OUT_S].T),             # [OUT_S, KT]
            "qzeros": np.ascontiguousarray(
                qzeros[:, c * zcols:(c + 1) * zcols]),
            "bias": np.ascontiguousarray(
                bias[o0:o0 + OUT_S].reshape(OUT_S // 128, 128).T),
        })
    return in_maps


def kernel(x, qweight, scales, qzeros, g_idx=None, bias=None, **_unused):
    """Full-input entry point: shards over 8 cores, runs on HW, gathers."""
    from concourse.bass_utils import run_bass_kernel_spmd

    nc = _get_program(SEQ, IN, OUT_S, CHUNK)
    in_maps = make_in_maps(x, qweight, scales, qzeros, bias)

    res = run_bass_kernel_spmd(nc, in_maps, core_ids=list(range(NCORES)))
    # each core returns outT [OUT_S, SEQ]; un-transpose + concat on host
    full = np.concatenate(
        [res.results[c]["out"].T for c in range(NCORES)], axis=1)
    return full.reshape(B, S, OUT).astype(np.float16)
